# revision 1
# baseline (speedup 1.0000x reference)
"""Two-layer LSTM generator, pure data parallelism across 8 NeuronCores.

Batch axis (4096) is sharded 8-ways (512/core); the ~2.9 MB of LSTM
weights are replicated on every core. No communication inside the time
loop. Shapes hardcoded per the problem spec: B=4096, H=300, T=64.
"""
import numpy as np
import jax
import jax.numpy as jnp
from jax import lax
from functools import partial

N_CORES = 8
H = 300


def _lstm_cell(x, h, c, W_ihT, W_hhT, b):
    gates = x @ W_ihT + h @ W_hhT + b  # [b, 4H]
    i = jax.nn.sigmoid(gates[:, 0 * H:1 * H])
    f = jax.nn.sigmoid(gates[:, 1 * H:2 * H])
    g = jnp.tanh(gates[:, 2 * H:3 * H])
    o = jax.nn.sigmoid(gates[:, 3 * H:4 * H])
    c_new = f * c + i * g
    h_new = o * jnp.tanh(c_new)
    return h_new, c_new


@partial(jax.pmap, axis_name="x", static_broadcasted_argnums=(7,),
         in_axes=(0, None, None, None, None, None, None),
         out_axes=0)
def _run_shard(z, W_ih0T, W_hh0T, b0, W_ih1T, W_hh1T, b1, T):
    b = z.shape[0]
    h0 = jnp.zeros((b, H), jnp.float32)
    c0 = jnp.zeros((b, H), jnp.float32)

    def step(carry, _):
        x, h0_, c0_, h1_, c1_ = carry
        h0n, c0n = _lstm_cell(x, h0_, c0_, W_ih0T, W_hh0T, b0)
        h1n, c1n = _lstm_cell(h0n, h1_, c1_, W_ih1T, W_hh1T, b1)
        return (h1n, h0n, c0n, h1n, c1n), h1n

    _, ys = lax.scan(step, (z, h0, c0, h0, c0), None, length=T)
    # ys: [T, b, H] -> [b, T, H]
    return jnp.transpose(ys, (1, 0, 2))


def kernel(z, W_ih0, W_hh0, b_ih0, b_hh0, W_ih1, W_hh1, b_ih1, b_hh1,
           sentence_len):
    T = int(sentence_len)
    B = z.shape[0]
    per = B // N_CORES

    z_sh = np.asarray(z, np.float32).reshape(N_CORES, per, H)
    W_ih0T = np.ascontiguousarray(np.asarray(W_ih0, np.float32).T)
    W_hh0T = np.ascontiguousarray(np.asarray(W_hh0, np.float32).T)
    W_ih1T = np.ascontiguousarray(np.asarray(W_ih1, np.float32).T)
    W_hh1T = np.ascontiguousarray(np.asarray(W_hh1, np.float32).T)
    b0 = np.asarray(b_ih0, np.float32) + np.asarray(b_hh0, np.float32)
    b1 = np.asarray(b_ih1, np.float32) + np.asarray(b_hh1, np.float32)

    ys = _run_shard(z_sh, W_ih0T, W_hh0T, b0, W_ih1T, W_hh1T, b1, T)
    out = np.asarray(ys).reshape(B, T, H)[:, None, :, :]
    return out.astype(np.float32)



# revision 21
# speedup vs baseline: 1127.4358x; 1127.4358x over previous
"""Two-layer LSTM generator on 8 TRN2 NeuronCores via a hand-written Bass/Tile kernel.

Data-parallel: batch 4096 -> 512 per core; LSTM weights replicated per core.

Per-core layout (B=512, H=300, T=64):
  State lives transposed: [hidden, batch] with hidden on partitions, so the
  recurrent matmuls need no per-step transposes.  gates^T [1200, 512] =
  W_packed @ [x; h], tiled as 10 M-tiles and 5 K-tiles (N=512, bf16).
  Gate rows are permuted per layer so that
    - every elementwise operand pair is partition-aligned, and
    - layer outputs land exactly in the K-tile slots the next matmuls read.
  The 60-row third chunks of each gate share "wide" tiles (T2, T7, k2) of
  124 partitions: lo chunk at partitions 0..59, hi chunk at 64..123 (engine
  APs need 32-aligned partition bases), rows 60..63 are zero-weight dummies.
  K-tile state buffers (shared by both layers):
    h1a=h1[0:120] h1b=h1[120:240] h0a=h0[0:120] h0b=h0[120:240]
    SH[0:60]=h1[240:300]  SH[64:124]=h0[240:300]
  Per step, h1 is PE-transposed back to [batch, hidden] and DMA'd to the
  output y[b, t, :].
"""
import numpy as np
import ml_dtypes

B, H, T_DEF = 4096, 300, 64
NC_CORES = 8
BPC = B // NC_CORES          # 512 batch per core
G4 = 4 * H                   # 1200 gate rows
NM = 10                      # M-tiles
NK = 5                       # K-tiles
MW = [120, 120, 124, 120, 120, 120, 120, 124, 120, 120]   # M-tile widths
KH = [120, 120, 124, 120, 120]                             # K-tile heights
MOFF = np.concatenate([[0], np.cumsum(MW)]).tolist()       # offsets into packed M
MP = sum(MW)                 # 1208 packed M columns
KP = sum(KH)                 # 604 packed K rows
NBT = BPC // 128             # 4 batch tiles of 128 for output transpose

LO = (0, 60)                 # partition sub-ranges of a wide tile
HI = (64, 124)
FULL = (0, 120)

_D4 = [-1, -1, -1, -1]       # dummy rows

# scheduling knobs (tuned via perf model)
MORDER = tuple(range(10))
KORDER = (3, 4, 0, 1, 2)
CHUNK_AT = {9: [0, 1, 2]}  # emit chunks after these morder positions

# blob layout (bf16, [124, BLOB_W]): weights L0,L1 | biases L0,L1 | z chunks
WOFF = 0
BOFF = 2 * NK * MP
ZOFF = BOFF + 2 * NM
BLOB_W = ZOFF + 3 * BPC


def _layout():
    """midx[L]: packed M position -> original gate row (in [i|f|g|o] concat
    order, 0..1199) or -1 (zero dummy).  kidx[L]: packed K position ->
    original contract index (x: 0..299, h: 300..599) or -1."""
    r = lambda a, b: list(range(a, b))
    m0 = (r(0, 120) + r(120, 240)                     # T0,T1: i
          + r(240, 300) + _D4 + r(540, 600)           # T2: i c3 | f c3
          + r(300, 420) + r(420, 540)                 # T3,T4: f
          + r(600, 720) + r(720, 840)                 # T5,T6: g
          + r(840, 900) + _D4 + r(1140, 1200)         # T7: g c3 | o c3
          + r(900, 1020) + r(1020, 1140))             # T8,T9: o
    m1 = (r(0, 120) + r(120, 240)
          + r(540, 600) + _D4 + r(240, 300)           # T2: f c3 | i c3
          + r(300, 420) + r(420, 540)
          + r(600, 720) + r(720, 840)
          + r(1140, 1200) + _D4 + r(840, 900)         # T7: o c3 | g c3
          + r(900, 1020) + r(1020, 1140))
    k0 = (r(0, 120) + r(120, 240)                     # k0,k1: x
          + r(240, 300) + _D4 + r(540, 600)           # k2: x c3 | h c3
          + r(300, 420) + r(420, 540))                # k3,k4: h
    k1 = (r(0, 120) + r(120, 240)
          + r(540, 600) + _D4 + r(240, 300)           # k2: h c3 | x c3
          + r(300, 420) + r(420, 540))
    return [np.array(m0), np.array(m1)], [np.array(k0), np.array(k1)]


# Activation plan: per layer, per M-tile, list of (prange, func),
# "s"=sigmoid "t"=tanh.  Wide tiles with one func run a single (0,124) op
# (dummy rows are exact zeros -> harmless).
W2 = (0, 124)
ACT_PLAN = [
    {0: [(FULL, "s")], 1: [(FULL, "s")], 2: [(W2, "s")], 3: [(FULL, "s")],
     4: [(FULL, "s")], 5: [(FULL, "t")], 6: [(FULL, "t")],
     7: [(LO, "t"), (HI, "s")], 8: [(FULL, "s")], 9: [(FULL, "s")]},
    {0: [(FULL, "s")], 1: [(FULL, "s")], 2: [(W2, "s")], 3: [(FULL, "s")],
     4: [(FULL, "s")], 5: [(FULL, "t")], 6: [(FULL, "t")],
     7: [(LO, "s"), (HI, "t")], 8: [(FULL, "s")], 9: [(FULL, "s")]},
]

# Gate chunk maps: per layer, gate -> [(m_tile, prange)] for chunks 0,1,2.
def _gate_chunks(L):
    c2i = HI if L else LO
    c2f = LO if L else HI
    c2g = HI if L else LO
    c2o = LO if L else HI
    return {
        "i": [(0, FULL), (1, FULL), (2, c2i)],
        "f": [(3, FULL), (4, FULL), (2, c2f)],
        "g": [(5, FULL), (6, FULL), (7, c2g)],
        "o": [(8, FULL), (9, FULL), (7, c2o)],
    }


def _pack_weights(W_ih, W_hh, b_ih, b_hh, L):
    midx, kidx = _layout()
    Wcat = np.concatenate([np.asarray(W_ih, np.float32),
                           np.asarray(W_hh, np.float32)], axis=1)  # [1200, 600]
    Wx = np.zeros((G4 + 1, 601), np.float32)
    Wx[:G4, :600] = Wcat
    Wp = Wx[np.ix_(midx[L], kidx[L])].T            # [604, 1208], -1 -> zero row/col
    Wpad = np.zeros((NK, 124, MP), np.float32)
    off = 0
    for k in range(NK):
        Wpad[k, :KH[k], :] = Wp[off:off + KH[k]]
        off += KH[k]
    b = np.concatenate([np.asarray(b_ih, np.float32)
                        + np.asarray(b_hh, np.float32), [0.0]])[midx[L]]  # [1208]
    bpad = np.zeros((124, NM), np.float32)
    for m in range(NM):
        bpad[:MW[m], m] = b[MOFF[m]:MOFF[m + 1]]
    return Wpad.astype(ml_dtypes.bfloat16), bpad


# ---------------------------------------------------------------------------
# Bass kernel builder
# ---------------------------------------------------------------------------

def _build_nc(T, reps=1):
    import concourse.bass as bass
    import concourse.mybir as mybir
    import concourse.tile as tile
    from concourse import bacc
    from concourse.masks import make_identity

    f32 = mybir.dt.float32
    bf16 = mybir.dt.bfloat16
    AF = mybir.ActivationFunctionType
    FN = {"s": AF.Sigmoid, "t": AF.Tanh}

    nc = bacc.Bacc()
    blob = nc.declare_dram_parameter("blob", [124, BLOB_W], bf16, isOutput=False)
    y = nc.declare_dram_parameter("y", [BPC, T, H], bf16, isOutput=True)

    with tile.TileContext(nc) as tc:
        with (
            tc.tile_pool(name="const", bufs=1) as constp,
            tc.tile_pool(name="state", bufs=1) as statep,
            tc.tile_pool(name="work", bufs=3) as workp,
            tc.tile_pool(name="tmp", bufs=4) as tmpp,
            tc.tile_pool(name="outp", bufs=6) as outp,
            tc.tile_pool(name="gpsum", bufs=5, space=bass.MemorySpace.PSUM) as gps,
            tc.tile_pool(name="tpsum", bufs=3, space=bass.MemorySpace.PSUM) as tps,
        ):
            # ---- constants: single blob DMA (keeps init proc fan-in tiny) ----
            bl = constp.tile([124, BLOB_W], bf16, name="bl", tag="bl")
            nc.sync.dma_start(bl[:], blob[:])
            wt = [bl[:, WOFF + L * NK * MP:WOFF + (L + 1) * NK * MP]
                  .rearrange("p (k m) -> p k m", k=NK) for L in range(2)]
            bbf = [bl[:, BOFF + L * NM:BOFF + (L + 1) * NM] for L in range(2)]
            b = [constp.tile([124, NM], f32, name=f"b{L}", tag=f"b{L}")
                 for L in range(2)]
            for L in range(2):
                nc.scalar.activation(b[L][:], bbf[L], AF.Copy)
            ident = constp.tile([128, 128], bf16, name="ident", tag="ident")
            make_identity(nc, ident[:])

            # ---- state ----
            h1a = statep.tile([128, BPC], bf16, name="h1a", tag="h1a")
            h1b = statep.tile([128, BPC], bf16, name="h1b", tag="h1b")
            h0a = statep.tile([120, BPC], bf16, name="h0a", tag="h0a")
            h0b = statep.tile([120, BPC], bf16, name="h0b", tag="h0b")
            sh = statep.tile([128, BPC], bf16, name="sh", tag="sh")
            zal = bl[:, ZOFF:ZOFF + 3 * BPC].rearrange("p (c n) -> p c n", c=3)
            za = zal[0:120, 0, :]
            zb = zal[0:120, 1, :]
            zc = zal[0:60, 2, :]
            c0a = statep.tile([120, BPC], f32, name="c0a", tag="c0a")
            c0b = statep.tile([120, BPC], f32, name="c0b", tag="c0b")
            c1a = statep.tile([120, BPC], f32, name="c1a", tag="c1a")
            c1b = statep.tile([120, BPC], f32, name="c1b", tag="c1b")
            cc = statep.tile([124, BPC], f32, name="cc", tag="cc")
            # cc[0:60]=c1[240:300], cc[64:124]=c0[240:300]

            for t_ in (h1a, h1b, h0a, h0b, sh):
                nc.vector.memset(t_[:], 0.0)
            for t_ in (c0a, c0b, c1a, c1b, cc):
                nc.vector.memset(t_[:], 0.0)


            h1a_r = h1a[0:120, :]
            h1b_r = h1b[0:120, :]
            sh_r = sh[0:124, :]
            rhs = [
                [h1a_r, h1b_r, sh_r, h0a, h0b],   # layer 0
                [h0a, h0b, sh_r, h1a_r, h1b_r],   # layer 1
            ]
            cmap = [
                [(c0a, FULL), (c0b, FULL), (cc, HI)],
                [(c1a, FULL), (c1b, FULL), (cc, LO)],
            ]
            hdest = [
                [(h0a, FULL), (h0b, FULL), (sh, HI)],
                [(h1a, FULL), (h1b, FULL), (sh, LO)],
            ]

            def sl(t_, pr):
                return t_[pr[0]:pr[1], :]

            for rep in range(reps):
              for t in range(T):
                for L in range(2):
                    # chunk ch's gates live in m-tiles: ch0 {0,3,5,8},
                    # ch1 {1,4,6,9}, ch2 {2,7}.  Emit those m-tiles first and
                    # the chunk's DVE chain as soon as its gates exist, so the
                    # next layer's first matmuls (korder: fresh k2 last) can
                    # start while this layer's tail is still computing.
                    morder = MORDER
                    korder = KORDER
                    gc = _gate_chunks(L)

                    shifted = {}

                    def emit_chunk(ch, gsb):
                        mi, pi = gc["i"][ch]
                        mf, pf = gc["f"][ch]
                        mg, pg = gc["g"][ch]
                        mo, po = gc["o"][ch]
                        ct, pc = cmap[L][ch]
                        hd, ph = hdest[L][ch]
                        if ch == 2:
                            # i2/g2 were DMA-shifted to the c base right
                            # after their ACTs (off the critical chain)
                            gi, pi = shifted[2], pc
                            gg, pg = shifted[7], pc
                        else:
                            gi, gg = gsb[mi], gsb[mg]
                        fc = tmpp.tile([124, BPC], f32, name="fc", tag="fc")
                        ig = tmpp.tile([124, BPC], f32, name="ig", tag="ig")
                        th = tmpp.tile([124, BPC], f32, name="th", tag="th")
                        nc.vector.tensor_mul(sl(fc, pc), sl(gsb[mf], pf), sl(ct, pc))
                        nc.vector.tensor_mul(sl(ig, pc if ch == 2 else pi),
                                             sl(gi, pi), sl(gg, pg))
                        nc.vector.tensor_add(sl(ct, pc), sl(fc, pc),
                                             sl(ig, pc if ch == 2 else pi))
                        nc.scalar.activation(sl(th, pc), sl(ct, pc), AF.Tanh)
                        nc.vector.tensor_mul(sl(hd, ph), sl(gsb[mo], po), sl(th, pc))

                    gsb = {}
                    for pos, m in enumerate(morder):
                        ps = gps.tile([124, BPC], f32, name="gps", tag="gps")
                        pv = ps[0:MW[m], :]
                        ms = slice(MOFF[m], MOFF[m + 1])
                        if L == 0 and t == 0 and rep == 0:
                            nc.tensor.matmul(pv, wt[0][0:120, 0, ms], za,
                                             start=True, stop=False)
                            nc.tensor.matmul(pv, wt[0][0:120, 1, ms], zb,
                                             start=False, stop=False)
                            nc.tensor.matmul(pv, wt[0][0:60, 2, ms], zc,
                                             start=False, stop=True)
                        else:
                            for j, k in enumerate(korder):
                                nc.tensor.matmul(pv, wt[L][0:KH[k], k, ms],
                                                 rhs[L][k][:],
                                                 start=(j == 0), stop=(j == NK - 1))
                        g_ = workp.tile([124, BPC], f32, name=f"g{m}", tag=f"g{m}")
                        for pr, fn in ACT_PLAN[L][m]:
                            nc.scalar.activation(
                                sl(g_, pr), sl(ps, pr), FN[fn],
                                bias=b[L][pr[0]:pr[1], m:m + 1])
                        gsb[m] = g_
                        if m in (2, 7):
                            # shift the i2 (m=2) / g2 (m=7) chunk to the
                            # c-base now, while the tail is still computing
                            src_r = LO if L == 0 else HI
                            dst_r = HI if L == 0 else LO
                            s_ = tmpp.tile([124, BPC], f32, name=f"s{m}",
                                           tag=f"s{m}")
                            nc.sync.dma_start(sl(s_, dst_r), sl(g_, src_r))
                            shifted[m] = s_
                        for ch in CHUNK_AT.get(pos, ()):
                            emit_chunk(ch, gsb)

                # output: PE-transpose h1 -> [batch, hidden], ACT evicts
                # psum -> sbuf, one DMA per batch tile.
                for bt in range(NBT):
                    bs = slice(bt * 128, (bt + 1) * 128)
                    pt = tps.tile([128, H], bf16, name="tps", tag="tps")
                    nc.tensor.transpose(pt[:, 0:120], h1a[0:120, bs],
                                        ident[0:120, 0:120])
                    nc.tensor.transpose(pt[:, 120:240], h1b[0:120, bs],
                                        ident[0:120, 0:120])
                    nc.tensor.transpose(pt[:, 240:300], sh[0:60, bs],
                                        ident[0:60, 0:60])
                    ob = outp.tile([128, H], bf16, name="ob", tag="ob")
                    nc.scalar.activation(ob[:], pt[:], AF.Copy)
                    nc.sync.dma_start(y[bs, t, :], ob[:])

    nc.compile()
    return nc


# ---------------------------------------------------------------------------
# Host wrapper
# ---------------------------------------------------------------------------

_CACHE = {}


def _get_nc(T, reps=1):
    key = (T, reps)
    if key not in _CACHE:
        _CACHE[key] = _build_nc(T, reps)
    return _CACHE[key]


def _pack_all(inputs):
    z = np.asarray(inputs["z"], np.float32)
    base = np.zeros((124, BLOB_W), np.float32)
    for L, (wi, wh, bi, bh) in enumerate(
        [(inputs["W_ih0"], inputs["W_hh0"], inputs["b_ih0"], inputs["b_hh0"]),
         (inputs["W_ih1"], inputs["W_hh1"], inputs["b_ih1"], inputs["b_hh1"])]):
        Wp, bp = _pack_weights(wi, wh, bi, bh, L)
        base[:, WOFF + L * NK * MP:WOFF + (L + 1) * NK * MP] = \
            Wp.astype(np.float32).transpose(1, 0, 2).reshape(124, NK * MP)
        base[:, BOFF + L * NM:BOFF + (L + 1) * NM] = bp
    in_maps = []
    for c in range(NC_CORES):
        blob = base.copy()
        zT = z[c * BPC:(c + 1) * BPC, :].T                   # [300, 512]
        z3 = np.zeros((124, 3, BPC), np.float32)
        z3[0:120, 0, :] = zT[0:120]
        z3[0:120, 1, :] = zT[120:240]
        z3[0:60, 2, :] = zT[240:300]
        blob[:, ZOFF:ZOFF + 3 * BPC] = z3.reshape(124, 3 * BPC)
        in_maps.append({"blob": blob.astype(ml_dtypes.bfloat16)})
    return in_maps


def _run(inputs, trace=False, reps=1):
    from concourse.bass_utils import run_bass_kernel_spmd

    T = int(inputs["sentence_len"])
    nc = _get_nc(T, reps)
    in_maps = _pack_all(inputs)
    res = run_bass_kernel_spmd(nc, in_maps, list(range(NC_CORES)), trace=trace)
    ys = np.stack([np.asarray(res.results[c]["y"]) for c in range(NC_CORES)],
                  axis=0)
    out = np.ascontiguousarray(ys.reshape(B, T, H)[:, None, :, :]).astype(np.float32)
    return out, res


def kernel(z, W_ih0, W_hh0, b_ih0, b_hh0, W_ih1, W_hh1, b_ih1, b_hh1,
           sentence_len):
    out, _ = _run(dict(z=z, W_ih0=W_ih0, W_hh0=W_hh0, b_ih0=b_ih0, b_hh0=b_hh0,
                       W_ih1=W_ih1, W_hh1=W_hh1, b_ih1=b_ih1, b_hh1=b_hh1,
                       sentence_len=sentence_len))
    return out
